# revision 10
# baseline (speedup 1.0000x reference)
"""Distributed multi-head attention (RoPE, non-causal) for 8 TRN2 NeuronCores.

Problem: B=2, S=2048, DIM=768, H=12, HEAD_DIM=64, f32 I/O.

Sharding: 24 (batch, head) pairs -> core c handles batch c//4 and heads
3*(c%4) .. 3*(c%4)+2.  Per core (bf16 matmuls, f32 PSUM):
  * The softmax exp is the #1 engine load (12.6M elems/core, ScalarE-only
    at 1 elem/cyc/partition).  It is split: heads 0/1 + the first head-2
    chunks go to ScalarE ACT-exp; the tail head-2 chunks go to the DVE via
    two chained custom-DVE ops evaluating a degree-7 polynomial
    p(x) ~= exp(x/8) on |x|<=24 (raw-score range is +-21.4; poly rel err
    1.2e-3, far under the bf16 noise floor).  Custom-DVE ops cannot read
    PSUM, so each DVE tile is staged by one DVE tensor_copy.
  * QKV projection emitted K-first, slab-by-slab, so scores j=0..3 and the
    first exp fire ~10us earlier than a bulk-load design; input DMAs are
    issued across 4 engine queues (none on ScalarE, which is the critical
    engine).
  * RoPE fused out of PSUM in bf16 (2-4x DVE modes): DVE copies PSUM->SBUF
    bf16, rotate_half = 32-row partition-swap DMA (sign folded in the sin
    table), sin-mul on DVE, cos-mul + add on GpSimd (SBUF-only engine).
  * scores: heads 0,1 on PE row-halves (tile_position (0,0)/(64,0));
    head 2 pairs even/odd key chunks via 2 partition-shift copies.
  * out^T accumulated via lhsT=[v | ones | pad] (M=128); softmax
    denominator lands on psum partition 64; per head: DVE den copy +
    reciprocal_approx_fast + GpSimd partition_broadcast + one DVE multiply
    straight out of PSUM.
  * Per 512-query block TWO AllGathers: heads01 [128,512] as soon as both
    finish, head2 [64,512] after -- so the kernel tail only waits on a
    65KB collective.  A tiny warm-up AllGather is issued at t~0 to absorb
    the one-time ~12us collective-engine init.  W_proj rows are permuted
    host-side to match the gathered [heads01-of-all-groups | head2s]
    channel order.
  * Output projection on the core's own 512-row slice via cond-predicated
    loads (identical SPMD program, flags input picks the block).
Host side only shards/permutes/casts inputs and concatenates the 8
output slices.
"""

import sys

sys.path.insert(0, "/opt/trn_rl_repo")

import numpy as np
import ml_dtypes

import concourse.bass as bass
import concourse.mybir as mybir
import concourse.tile as tile
from concourse import bacc, bass_utils

BF16 = mybir.dt.bfloat16
F32 = mybir.dt.float32
AF = mybir.ActivationFunctionType

B, S, DIM, H, DH = 2, 2048, 768, 12, 64
THETA = 10000.0
N_CORES = 8
GROUPS = [[0, 1, 2, 3], [4, 5, 6, 7]]
HL = 3           # heads per core
SC = S // 4      # per-core output row slice (512)
KC = DIM // 128  # 6 contraction chunks
NJ = S // 128    # 16 key chunks
NB = 4           # query blocks per core batch

# head-2 exp chunk-pairs per block routed to the DVE poly-exp (0..8)
DVE_PAIRS = 4

_CACHED = {}

# --------------------------------------------------------------------------
# Custom DVE exp(x/8): degree-7 poly, p(0)=1, fitted minimax-relative on
# |x|<=24 (raw scores span +-21.4).  Split into two 7-stage ops:
#   PART1: H = (((c7*x + c6)*x + c5)*x + c4)*x        (c4 spilled via in1)
#   PART2: out = ((((H + c3)*x + c2)*x + c1)*x) + 1
_EXP_C = [
    1.25124039e-01, 7.79104663e-03, 3.19739592e-04, 1.00846613e-05,
    2.82333267e-07, 6.39666992e-09, 7.30440201e-11,
]


def _register_exp_ops():
    if "ops" in _CACHED:
        return _CACHED["ops"]
    import concourse.dve_ops as dve_ops
    from concourse.dve_ops import DveOp
    from concourse.dve_spec import C0, C1, C2, C3, One, Spec, Src0, Src1, \
        _spill_c3_to_src1, lower
    from concourse.dve_uop import DveOpSpec

    def ref1(in0, in1, c0, c1, c2):
        x = in0.astype(np.float32)
        return ((((c0 * x + c1) * x + c2) * x + in1) * x).astype(np.float32)

    def ref2(in0, in1, c0, c1, c2):
        x = in0.astype(np.float32)
        h = in1.astype(np.float32)
        return (((h + c0) * x + c1) * x + c2) * x + np.float32(1.0)

    body1 = _spill_c3_to_src1((((Src0 * C0 + C1) * Src0 + C2) * Src0 + C3) * Src0)
    body2 = (((Src1 + C0) * Src0 + C1) * Src0 + C2) * Src0 + One

    def mk(name, body, ref):
        if name in dve_ops._SUB_OPCODE_FOR_NAME:
            return next(o for o in dve_ops.OPS if o.name == name)
        spec = Spec(body=body, reference=ref)
        shas = {}
        for ver in ("v3", "v4"):
            shas[ver] = DveOpSpec(
                name=name, opcode=31, uops=lower(spec, ver=ver), rd1_en=True
            ).sha(ver)
        op = DveOp(name, spec, subdim=False, uops_sha=shas)
        dve_ops.OPS.append(op)
        dve_ops.CUSTOM_DVE_SPECS[name] = spec
        dve_ops._SUB_OPCODE_FOR_NAME[name] = (
            max(dve_ops._SUB_OPCODE_FOR_NAME.values()) + 1
        )
        assert max(dve_ops._SUB_OPCODE_FOR_NAME.values()) < 0x20
        return op

    e1 = mk("EXP8_PART1_ANT", body1, ref1)
    e2 = mk("EXP8_PART2_ANT", body2, ref2)
    _CACHED["ops"] = (e1, e2)
    return e1, e2


def _build():
    """Build the SPMD Bacc graph (identical on all 8 cores)."""
    EXP1, EXP2 = _register_exp_ops()
    c1, c2, c3, c4, c5, c6, c7 = [float(np.float32(v)) for v in _EXP_C]

    nc = bacc.Bacc(None, target_bir_lowering=False)

    xT = nc.declare_dram_parameter("xT", [DIM, S], BF16, isOutput=False)
    wqk = nc.declare_dram_parameter("wqk", [DIM, 2 * HL * DH], BF16, isOutput=False)
    wv = nc.declare_dram_parameter("wv", [DIM, HL * DH], BF16, isOutput=False)
    cosq = nc.declare_dram_parameter("cosq", [128, S], BF16, isOutput=False)
    sinq = nc.declare_dram_parameter("sinq", [128, S], BF16, isOutput=False)
    wp = nc.declare_dram_parameter("wp", [DIM, DIM], BF16, isOutput=False)
    bp = nc.declare_dram_parameter("bp", [1, DIM], F32, isOutput=False)
    flags = nc.declare_dram_parameter("flags", [1, NB], mybir.dt.uint32,
                                      isOutput=False)
    out_d = nc.declare_dram_parameter("out", [SC, DIM], F32, isOutput=True)

    scale = DH ** -0.5

    with tile.TileContext(nc) as tc:
        with (
            tc.tile_pool(name="const", bufs=1) as const,
            tc.tile_pool(name="work", bufs=2) as work,
            tc.tile_pool(name="psum", bufs=2, space="PSUM") as psum,
            tc.tile_pool(name="dram", bufs=1, space="DRAM") as dram,
        ):
            # ---- input loads, spread over 4 non-ACT queues ----------------
            xT_sb = const.tile([128, KC, S], BF16)
            wqk_sb = const.tile([128, KC, 2 * HL * DH], BF16)
            wv_sb = const.tile([128, KC, HL * DH], BF16)
            wp_sb = const.tile([128, KC, DIM], BF16)
            cos_sb = const.tile([128, S], BF16)
            sin_sb = const.tile([128, S], BF16)
            bp_sb = const.tile([1, DIM], F32)

            # ---- per-block predication flags FIRST: tile_critical quiesces
            # prior queue activity, so it must precede every DMA issue
            with tc.tile_critical():
                conds = []
                for i in range(NB):
                    r = nc.gpsimd.alloc_register(f"flag_{i}")
                    nc.gpsimd.reg_load(r, flags[0:1, i:i + 1])
                    conds.append(nc.gpsimd.snap(r, donate=True, min_val=0,
                                                max_val=1))

            ones128 = const.tile([128, 128], F32)
            nc.vector.memset(ones128[:], 1.0)
            c4b = const.tile([128, 1], F32)
            nc.vector.memset(c4b[:], c4)

            # preload the exp table set while ScalarE is idle
            warm_sb = work.tile([1, 16], F32, tag="warm")
            nc.scalar.activation(warm_sb[:], ones128[0:1, 0:16], AF.Exp)

            # critical path: wqk + xT slab 0 (block-0 scores j=0..3), split
            # across the SP and ACT DGE queues (ACT is idle until ~11us)
            for k in range(3):
                nc.sync.dma_start(wqk_sb[:, k, :], wqk[k * 128:(k + 1) * 128, :])
            nc.scalar.dma_start(cos_sb[:, 0:1024], cosq[:, 0:1024])
            nc.scalar.dma_start(sin_sb[:, 0:1024], sinq[:, 0:1024])
            for k in range(3):
                nc.scalar.dma_start(wqk_sb[:, k + 3, :],
                                    wqk[(k + 3) * 128:(k + 4) * 128, :])
            for k in range(3):
                nc.sync.dma_start(xT_sb[:, k, 0:512],
                                  xT[k * 128:(k + 1) * 128, 0:512])
                nc.scalar.dma_start(xT_sb[:, k + 3, 0:512],
                                    xT[(k + 3) * 128:(k + 4) * 128, 0:512])

            # ---- collective warm-up (absorbs the barrier + cc init) -------
            warm_in = dram.tile([64, 16], BF16, tag="warm_in")
            warm_out = dram.tile([256, 16], BF16, tag="warm_out")
            nc.gpsimd.collective_compute(
                "AllGather", mybir.AluOpType.bypass, replica_groups=GROUPS,
                ins=[warm_in[:].opt()], outs=[warm_out[:]],
            )

            # remaining xT slabs + tables + weights (non-critical)
            for sb in range(1, 4):
                sl = slice(sb * 512, (sb + 1) * 512)
                for k in range(KC):
                    q = nc.sync if k < 3 else nc.gpsimd
                    q.dma_start(xT_sb[:, k, sl], xT[k * 128:(k + 1) * 128, sl])
            nc.scalar.dma_start(cos_sb[:, 1024:2048], cosq[:, 1024:2048])
            nc.scalar.dma_start(sin_sb[:, 1024:2048], sinq[:, 1024:2048])
            for k in range(KC):
                nc.scalar.dma_start(wv_sb[:, k, :], wv[k * 128:(k + 1) * 128, :])
            nc.scalar.dma_start(bp_sb[:], bp[:])
            for k in range(KC):
                (nc.gpsimd if k % 2 else nc.sync).dma_start(
                    wp_sb[:, k, :], wp[k * 128:(k + 1) * 128, :])

            # ---- qk^T = wqk.T @ xT with fused RoPE ------------------------
            # wqk column order [q0, q1 | k0, k1 | q2, k2], channels
            # deinterleaved per head so rotate_half is a 32-partition swap.
            qkb = const.tile([128, 3, S], BF16)
            # head-2 partition-shift copies: qk2d = [k2 (lo) | q2 (hi)]
            qk2d = const.tile([128, S], BF16)

            def emit_qk_tile(mb, sb, dve_all=False):
                sl = slice(sb * 512, (sb + 1) * 512)
                ps = psum.tile([128, 512], F32, tag="ps_mm", bufs=1)
                for k in range(KC):
                    nc.tensor.matmul(
                        ps[:],
                        wqk_sb[:, k, mb * 128:(mb + 1) * 128],
                        xT_sb[:, k, sl],
                        start=(k == 0), stop=(k == KC - 1),
                    )
                qks = work.tile([128, 512], BF16, tag="qks", bufs=3)
                nc.vector.tensor_copy(qks[:], ps[:])
                rot = work.tile([128, 512], BF16, tag="rot", bufs=3)
                for g in range(2):
                    o = g * 64
                    nc.sync.dma_start(rot[o:o + 32, :], qks[o + 32:o + 64, :])
                    nc.sync.dma_start(rot[o + 32:o + 64, :], qks[o:o + 32, :])
                rots = work.tile([128, 512], BF16, tag="rots", bufs=2)
                nc.vector.tensor_mul(rots[:], rot[:], sin_sb[:, sl])
                tmp = work.tile([128, 512], BF16, tag="tmp", bufs=2)
                e = nc.vector if dve_all else nc.gpsimd
                e.tensor_mul(tmp[:], qks[:], cos_sb[:, sl])
                e.tensor_add(qkb[:, mb, sl], tmp[:], rots[:])
                if mb == 2:
                    # per-slab head-2 swap, so sc2 never waits on all slabs
                    nc.sync.dma_start(qk2d[0:64, sl], qkb[64:128, 2, sl])
                    nc.sync.dma_start(qk2d[64:128, sl], qkb[0:64, 2, sl])

            # k01 slab 0 then q01 slab 0 unblocks scores j=0-3 early
            emit_qk_tile(1, 0, dve_all=True)
            emit_qk_tile(0, 0, dve_all=True)

            # v in [keys, ch]; slab per head = [v | ones | pad]; the memset-1
            # leaves pad columns at 1.0 (harmless extra denominator copies in
            # unread psum rows 65-127)
            v_aug = const.tile([128, NJ, HL * 128], BF16)
            for h in range(HL):
                nc.vector.memset(v_aug[:, :, h * 128 + DH:(h + 1) * 128], 1.0)

            def emit_v_chunk(st):
                ps = psum.tile([128, HL * DH], F32, tag="ps_mm", bufs=1)
                for k in range(KC):
                    nc.tensor.matmul(
                        ps[:],
                        xT_sb[:, k, st * 128:(st + 1) * 128],
                        wv_sb[:, k, :],
                        start=(k == 0), stop=(k == KC - 1),
                    )
                dst = v_aug[:, st, :].rearrange(
                    "p (h x) -> p h x", h=HL)[:, :, 0:DH]
                src = ps.rearrange("p (h x) -> p h x", h=HL)
                nc.vector.tensor_copy(dst, src)

            bp128 = const.tile([128, DIM], F32)

            def emit_bias_bcast():
                for o0, on in ((0, 512), (512, 256)):
                    psb = psum.tile([128, on], F32, tag="ps_mm", bufs=1)
                    nc.tensor.matmul(
                        psb[:], ones128[0:1, :], bp_sb[0:1, o0:o0 + on],
                        start=True, stop=True,
                    )
                    nc.vector.tensor_copy(bp128[:, o0:o0 + on], psb[:])

            # ---- attention ------------------------------------------------
            agZ1, agZ2 = [], []
            for b in range(NB):
                agZ1.append(dram.tile([512, SC], BF16, tag=f"agZ1_{b}",
                                      name=f"agZ1_{b}"))
                agZ2.append(dram.tile([256, SC], BF16, tag=f"agZ2_{b}",
                                      name=f"agZ2_{b}"))
            P01 = const.tile([128, NJ, 2, 512], BF16)
            P2 = [const.tile([128, NJ, 512], BF16, tag=f"P2_{i}", name=f"P2_{i}")
                  for i in range(2)]

            def emit_dve_exp(ps2, dst):
                ssb = work.tile([128, 2, 512], F32, tag="ssb", bufs=2)
                nc.vector.tensor_copy(ssb[:], ps2[:])
                hh = work.tile([128, 2, 512], F32, tag="hh", bufs=2)
                sflat = ssb[:].rearrange("p a b -> p (a b)")
                hflat = hh[:].rearrange("p a b -> p (a b)")
                nc.vector._custom_dve(
                    EXP1, out=hflat, in0=sflat, in1=c4b[:],
                    s0=c7, s1=c6, imm2=c5,
                )
                nc.vector._custom_dve(
                    EXP2, out=dst, in0=sflat, in1=hflat,
                    s0=c3, s1=c2, imm2=c1,
                )

            def emit_norm(ps_o, dst_d, r0, tag):
                # denominator sits on psum partition 64; copy to SBUF first
                # (custom-DVE reciprocal must NOT read PSUM), broadcast 1/den
                # across partitions on GpSimd, multiply straight out of PSUM
                den = work.tile([1, 512], F32, tag="den")
                nc.vector.tensor_copy(den[:], ps_o[64:65, :])
                rcp = work.tile([1, 512], F32, tag="rcp")
                nc.vector.reciprocal_approx_fast(rcp[:], den[:])
                rcpb = work.tile([DH, 512], F32, tag="rcpb")
                nc.gpsimd.partition_broadcast(rcpb[:], rcp[:], channels=DH)
                ob = work.tile([DH, 512], BF16, tag="ob", bufs=3)
                nc.vector.tensor_mul(ob[:], ps_o[0:DH, :], rcpb[:])
                nc.sync.dma_start(dst_d[r0:r0 + DH, :], ob[:])

            # DVE head-2 pairs per block (tail chunks of the t loop)
            DP = [0, 4, 4, 2]
            ob1_ds = [dram.tile([128, SC], BF16, tag=f"ob1_{b}",
                                name=f"ob1_{b}") for b in range(NB)]
            ob2_ds = [dram.tile([DH, SC], BF16, tag=f"ob2_{b}",
                                name=f"ob2_{b}") for b in range(NB)]

            def emit_sc01(b, j):
                isl = slice(b * 512, (b + 1) * 512)
                ps2 = psum.tile([128, 2, 512], F32, tag="ps_s")
                nc.tensor.matmul(
                    ps2[:, 0, :],
                    qkb[0:64, 1, j * 128:(j + 1) * 128],
                    qkb[0:64, 0, isl], start=True, stop=True,
                    tile_position=(0, 0),
                )
                nc.tensor.matmul(
                    ps2[:, 1, :],
                    qkb[64:128, 1, j * 128:(j + 1) * 128],
                    qkb[64:128, 0, isl], start=True, stop=True,
                    tile_position=(64, 0),
                )
                nc.scalar.activation(
                    P01[:, j, :, :], ps2[:], AF.Exp, scale=scale
                )

            def emit_sc2(b, t):
                isl = slice(b * 512, (b + 1) * 512)
                j0, j1 = 2 * t, 2 * t + 1
                P2b = P2[b % 2]
                ps2 = psum.tile([128, 2, 512], F32, tag="ps_s")
                nc.tensor.matmul(
                    ps2[:, 0, :],
                    qk2d[0:64, j0 * 128:(j0 + 1) * 128],
                    qkb[0:64, 2, isl], start=True, stop=True,
                    tile_position=(0, 0),
                )
                nc.tensor.matmul(
                    ps2[:, 1, :],
                    qkb[64:128, 2, j1 * 128:(j1 + 1) * 128],
                    qk2d[64:128, isl], start=True, stop=True,
                    tile_position=(64, 0),
                )
                if t < NJ // 2 - DP[b]:
                    nc.scalar.activation(
                        P2b[:, j0:j0 + 2, :], ps2[:], AF.Exp, scale=scale
                    )
                else:
                    emit_dve_exp(ps2, P2b[:, j0:j0 + 2, :])

            def emit_pv2_chunks(b, jcs, ps_o2):
                for jc in jcs:
                    nc.tensor.matmul(
                        ps_o2[:],
                        v_aug[:, jc, 2 * 128:3 * 128],
                        P2[b % 2][:, jc, :],
                        start=(jc == 0), stop=(jc == NJ - 1),
                    )

            def emit_gather(src, dst):
                nc.gpsimd.collective_compute(
                    "AllGather", mybir.AluOpType.bypass, replica_groups=GROUPS,
                    ins=[src[:].opt()], outs=[dst[:]],
                )

            # Software pipeline: stageA(b) interleaves block b-1's head-2
            # scores (+ its pv2 chunks trailing 2 pairs behind) with block
            # b's heads01 scores j=0..7; stageB(b) runs j=8..15 with both
            # pv01 accumulations trailing 2 chunks/iter, finishes block b-1's
            # pv2 + gather2, then norms + gather1 for block b.  This keeps
            # ScalarE exp-fed continuously across block boundaries.
            ps_o2_prev = None
            for b in range(NB):
                # ---------------- stageA ----------------
                if b >= 1:
                    ps_o2_prev = psum.tile([128, 512], F32, tag="ps_p2", bufs=1)
                for t in range(8):
                    if b >= 1:
                        emit_sc2(b - 1, t)
                    emit_sc01(b, t)
                    if b >= 1 and t >= 2:
                        emit_pv2_chunks(b - 1, (2 * (t - 2), 2 * t - 3),
                                        ps_o2_prev)
                    if b == 0:
                        if t == 0:
                            emit_qk_tile(1, 1)
                        elif t == 1:
                            emit_bias_bcast()
                        elif t in (2, 3):
                            emit_v_chunk(2 * (t - 2))
                            emit_v_chunk(2 * t - 3)
                        elif t == 4:
                            emit_qk_tile(1, 2)
                        elif t in (5, 7):
                            emit_v_chunk(t - 1)
                            emit_v_chunk(t)
                        elif t == 6:
                            emit_qk_tile(1, 3)
                    elif b == 1 and t == 0:
                        emit_qk_tile(0, 2)
                    elif b == 2 and t == 0:
                        emit_qk_tile(0, 3)
                # ---------------- stageB ----------------
                ps_o0 = psum.tile([128, 512], F32, tag="ps_o")
                ps_o1 = psum.tile([128, 512], F32, tag="ps_o")
                for j in range(8, 16):
                    emit_sc01(b, j)
                    if b == 0:
                        if j in (8, 10, 12, 14):
                            emit_qk_tile(2, (j - 8) // 2)
                        elif j == 9:
                            emit_v_chunk(8)
                            emit_v_chunk(9)
                        elif j == 11:
                            emit_v_chunk(10)
                            emit_v_chunk(11)
                        elif j == 13:
                            emit_v_chunk(12)
                            emit_v_chunk(13)
                            emit_qk_tile(0, 1)
                        elif j == 15:
                            emit_v_chunk(14)
                            emit_v_chunk(15)
                    for jc in (2 * (j - 8), 2 * j - 15):
                        for h, pso in ((0, ps_o0), (1, ps_o1)):
                            nc.tensor.matmul(
                                pso[:],
                                v_aug[:, jc, h * 128:(h + 1) * 128],
                                P01[:, jc, h, :],
                                start=(jc == 0), stop=(jc == NJ - 1),
                            )
                    if b >= 1:
                        if j == 8:
                            emit_pv2_chunks(b - 1, (12, 13), ps_o2_prev)
                        elif j == 9:
                            emit_pv2_chunks(b - 1, (14, 15), ps_o2_prev)
                            emit_norm(ps_o2_prev, ob2_ds[b - 1], 0, f"{b-1}_2")
                            emit_gather(ob2_ds[b - 1], agZ2[b - 1])
                emit_norm(ps_o0, ob1_ds[b], 0, f"{b}_0")
                emit_norm(ps_o1, ob1_ds[b], DH, f"{b}_1")
                emit_gather(ob1_ds[b], agZ1[b])

            # epilogue: block 3 head-2 scores + pv2 + final small gather
            ps_o2_prev = psum.tile([128, 512], F32, tag="ps_p2", bufs=1)
            for t in range(8):
                emit_sc2(3, t)
                if t >= 2:
                    emit_pv2_chunks(3, (2 * (t - 2), 2 * t - 3), ps_o2_prev)
            emit_pv2_chunks(3, (12, 13, 14, 15), ps_o2_prev)
            emit_norm(ps_o2_prev, ob2_ds[3], 0, "3_2")
            emit_gather(ob2_ds[3], agZ2[3])

            # keep the PE's HAM window busy while the last gather lands
            for w in range(8):
                wps = psum.tile([128, 512], F32, tag="ps_p2", bufs=1)
                nc.tensor.matmul(
                    wps[:], qkb[:, 0, 0:128], qkb[:, 1, 0:512],
                    start=True, stop=True,
                )

            # ---- output projection on my 512-row slice --------------------
            # cond-predicated loads: only block g's flag is 1 on core g
            ag1_sb = const.tile([128, 4, SC], BF16)
            ag2_sb = const.tile([128, 2, SC], BF16)
            for b in range(NB):
                nc.gpsimd.dma_start(
                    ag1_sb[:],
                    agZ1[b][:].rearrange("(k p) n -> p k n", p=128),
                    cond=conds[b],
                )
            for b in range(NB):
                nc.gpsimd.dma_start(
                    ag2_sb[:],
                    agZ2[b][:].rearrange("(k p) n -> p k n", p=128),
                    cond=conds[b],
                )

            for m in range(SC // 128):
                for o0, on in ((0, 512), (512, 256)):
                    ps_p = psum.tile([128, on], F32, tag="ps_o")
                    for k in range(4):
                        nc.tensor.matmul(
                            ps_p[:],
                            ag1_sb[:, k, m * 128:(m + 1) * 128],
                            wp_sb[:, k, o0:o0 + on],
                            start=(k == 0), stop=False,
                        )
                    for k2 in range(2):
                        nc.tensor.matmul(
                            ps_p[:],
                            ag2_sb[:, k2, m * 128:(m + 1) * 128],
                            wp_sb[:, 4 + k2, o0:o0 + on],
                            start=False, stop=(k2 == 1),
                        )
                    po = work.tile([128, on], F32, tag="po", bufs=4)
                    nc.vector.tensor_add(po[:], ps_p[:], bp128[:, o0:o0 + on])
                    (nc.sync if m % 2 else nc.gpsimd).dma_start(
                        out_d[m * 128:(m + 1) * 128, o0:o0 + on], po[:]
                    )

    nc.compile()
    return nc


def _rope_tables():
    bf16 = ml_dtypes.bfloat16
    inv = (1.0 / (THETA ** (np.arange(0, DH, 2, dtype=np.float32) / DH))).astype(
        np.float32
    )
    pos = np.arange(S, dtype=np.float32)
    f = pos[:, None] * inv[None, :]           # [S, 32] f32, matches reference
    c = np.cos(f).T.astype(np.float32)        # [32, S]
    s = np.sin(f).T.astype(np.float32)
    cos64 = np.concatenate([c, c], axis=0)    # rows i and 32+i = cos(f_i)
    sin64 = np.concatenate([-s, s], axis=0)   # sign folded for rotate_half
    return (
        np.concatenate([cos64, cos64], axis=0).astype(bf16),  # [128, S]
        np.concatenate([sin64, sin64], axis=0).astype(bf16),
    )


def _shard_inputs(x, W_qkv, W_proj, b_proj):
    bf16 = ml_dtypes.bfloat16
    cos128, sin128 = _rope_tables()
    # deinterleave perm: new[i] = orig[2i] (i<32), new[32+i] = orig[2i+1]
    perm = np.concatenate([np.arange(0, DH, 2), np.arange(1, DH, 2)])
    # W_proj input-channel order after the split gathers:
    #   rows 0..511:  group g, heads {0,1}:  idx 128g + 64h + d
    #   rows 512..767: group g, head 2:      idx 512 + 64g + d
    ch = np.empty(DIM, dtype=np.int64)
    for g in range(4):
        for h in range(2):
            ch[128 * g + 64 * h: 128 * g + 64 * h + 64] = \
                (HL * g + h) * DH + np.arange(DH)
        ch[512 + 64 * g: 512 + 64 * g + 64] = (HL * g + 2) * DH + np.arange(DH)
    wp_t = np.ascontiguousarray(W_proj.T[ch]).astype(bf16)      # [c_perm, o]
    bp_r = np.ascontiguousarray(b_proj[None, :]).astype(np.float32)
    in_maps = []
    for c in range(N_CORES):
        b, g = c // 4, c % 4
        hs = [HL * g + i for i in range(HL)]
        q_r = [h * DH + perm for h in hs]
        k_r = [DIM + h * DH + perm for h in hs]
        # column order [q0, q1 | k0, k1 | q2, k2] to align base partitions
        qk_rows = np.concatenate([q_r[0], q_r[1], k_r[0], k_r[1], q_r[2], k_r[2]])
        v_rows = np.concatenate([2 * DIM + h * DH + np.arange(DH) for h in hs])
        flag = np.zeros(NB, dtype=np.uint32)
        flag[g] = 1
        in_maps.append({
            "xT": np.ascontiguousarray(x[b].T).astype(bf16),
            "wqk": np.ascontiguousarray(W_qkv[qk_rows].T).astype(bf16),
            "wv": np.ascontiguousarray(W_qkv[v_rows].T).astype(bf16),
            "cosq": cos128,
            "sinq": sin128,
            "wp": wp_t,
            "bp": bp_r,
            "flags": flag[None, :],
        })
    return in_maps


def run(inputs, trace=False, tmpdir=None):
    if "nc" not in _CACHED:
        _CACHED["nc"] = _build()
    nc = _CACHED["nc"]
    in_maps = _shard_inputs(
        inputs["x"], inputs["W_qkv"], inputs["W_proj"], inputs["b_proj"]
    )
    res = bass_utils.run_bass_kernel_spmd(
        nc, in_maps, core_ids=list(range(N_CORES)), trace=trace, tmpdir=tmpdir
    )
    out = np.empty((B, S, DIM), dtype=np.float32)
    for c in range(N_CORES):
        b, g = c // 4, c % 4
        out[b, g * SC:(g + 1) * SC, :] = res.results[c]["out"]
    return out, res


def kernel(**inputs):
    out, _ = run(inputs, trace=False)
    return out


# revision 13
# speedup vs baseline: 1.3578x; 1.3578x over previous
"""Distributed multi-head attention (RoPE, non-causal) for 8 TRN2 NeuronCores.

Problem: B=2, S=2048, DIM=768, H=12, HEAD_DIM=64, f32 I/O.

Sharding: 24 (batch, head) pairs -> core c handles batch c//4 and heads
3*(c%4) .. 3*(c%4)+2.  Per core (bf16 matmuls, f32 PSUM):
  * The softmax exp is the #1 engine load (12.6M elems/core, ScalarE-only
    at 1 elem/cyc/partition).  It is split: heads 0/1 + the first head-2
    chunks go to ScalarE ACT-exp; the tail head-2 chunks go to the DVE via
    two chained custom-DVE ops evaluating a degree-7 polynomial
    p(x) ~= exp(x/8) on |x|<=24 (raw-score range is +-21.4; poly rel err
    1.2e-3, far under the bf16 noise floor).  Custom-DVE ops cannot read
    PSUM, so each DVE tile is staged by one DVE tensor_copy.
  * QKV projection emitted K-first, slab-by-slab, so scores j=0..3 and the
    first exp fire ~10us earlier than a bulk-load design; input DMAs are
    issued across 4 engine queues (none on ScalarE, which is the critical
    engine).
  * RoPE fused out of PSUM in bf16 (2-4x DVE modes): DVE copies PSUM->SBUF
    bf16, rotate_half = 32-row partition-swap DMA (sign folded in the sin
    table), sin-mul on DVE, cos-mul + add on GpSimd (SBUF-only engine).
  * scores: heads 0,1 on PE row-halves (tile_position (0,0)/(64,0));
    head 2 pairs even/odd key chunks via 2 partition-shift copies.
  * out^T accumulated via lhsT=[v | ones | pad] (M=128); softmax
    denominator lands on psum partition 64; per head: DVE den copy +
    reciprocal_approx_fast + GpSimd partition_broadcast + one DVE multiply
    straight out of PSUM.
  * Per 512-query block TWO AllGathers: heads01 [128,512] as soon as both
    finish, head2 [64,512] after -- so the kernel tail only waits on a
    65KB collective.  A tiny warm-up AllGather is issued at t~0 to absorb
    the one-time ~12us collective-engine init.  W_proj rows are permuted
    host-side to match the gathered [heads01-of-all-groups | head2s]
    channel order.
  * Output projection on the core's own 512-row slice via cond-predicated
    loads (identical SPMD program, flags input picks the block).
Host side only shards/permutes/casts inputs and concatenates the 8
output slices.
"""

import sys

sys.path.insert(0, "/opt/trn_rl_repo")

import numpy as np
import ml_dtypes

import concourse.bass as bass
import concourse.mybir as mybir
import concourse.tile as tile
from concourse import bacc, bass_utils

BF16 = mybir.dt.bfloat16
F32 = mybir.dt.float32
AF = mybir.ActivationFunctionType

B, S, DIM, H, DH = 2, 2048, 768, 12, 64
THETA = 10000.0
N_CORES = 8
GROUPS = [[0, 1, 2, 3], [4, 5, 6, 7]]
HL = 3           # heads per core
SC = S // 4      # per-core output row slice (512)
KC = DIM // 128  # 6 contraction chunks
NJ = S // 128    # 16 key chunks
NB = 4           # query blocks per core batch

# head-2 exp chunk-pairs per block routed to the DVE poly-exp (0..8)
DVE_PAIRS = 4

_CACHED = {}

# --------------------------------------------------------------------------
# Custom DVE exp(x/8): degree-7 poly, p(0)=1, fitted minimax-relative on
# |x|<=24 (raw scores span +-21.4).  Split into two 7-stage ops:
#   PART1: H = (((c7*x + c6)*x + c5)*x + c4)*x        (c4 spilled via in1)
#   PART2: out = ((((H + c3)*x + c2)*x + c1)*x) + 1
_EXP_C = [
    1.25124039e-01, 7.79104663e-03, 3.19739592e-04, 1.00846613e-05,
    2.82333267e-07, 6.39666992e-09, 7.30440201e-11,
]


def _register_exp_ops():
    if "ops" in _CACHED:
        return _CACHED["ops"]
    import concourse.dve_ops as dve_ops
    from concourse.dve_ops import DveOp
    from concourse.dve_spec import C0, C1, C2, C3, One, Spec, Src0, Src1, \
        _spill_c3_to_src1, lower
    from concourse.dve_uop import DveOpSpec

    def ref1(in0, in1, c0, c1, c2):
        x = in0.astype(np.float32)
        return ((((c0 * x + c1) * x + c2) * x + in1) * x).astype(np.float32)

    def ref2(in0, in1, c0, c1, c2):
        x = in0.astype(np.float32)
        h = in1.astype(np.float32)
        return (((h + c0) * x + c1) * x + c2) * x + np.float32(1.0)

    body1 = _spill_c3_to_src1((((Src0 * C0 + C1) * Src0 + C2) * Src0 + C3) * Src0)
    body2 = (((Src1 + C0) * Src0 + C1) * Src0 + C2) * Src0 + One

    def mk(name, body, ref):
        if name in dve_ops._SUB_OPCODE_FOR_NAME:
            return next(o for o in dve_ops.OPS if o.name == name)
        spec = Spec(body=body, reference=ref)
        shas = {}
        for ver in ("v3", "v4"):
            shas[ver] = DveOpSpec(
                name=name, opcode=31, uops=lower(spec, ver=ver), rd1_en=True
            ).sha(ver)
        op = DveOp(name, spec, subdim=False, uops_sha=shas)
        dve_ops.OPS.append(op)
        dve_ops.CUSTOM_DVE_SPECS[name] = spec
        dve_ops._SUB_OPCODE_FOR_NAME[name] = (
            max(dve_ops._SUB_OPCODE_FOR_NAME.values()) + 1
        )
        assert max(dve_ops._SUB_OPCODE_FOR_NAME.values()) < 0x20
        return op

    e1 = mk("EXP8_PART1_ANT", body1, ref1)
    e2 = mk("EXP8_PART2_ANT", body2, ref2)
    _CACHED["ops"] = (e1, e2)
    return e1, e2


def _build():
    """Build the SPMD Bacc graph (identical on all 8 cores)."""
    EXP1, EXP2 = _register_exp_ops()
    c1, c2, c3, c4, c5, c6, c7 = [float(np.float32(v)) for v in _EXP_C]

    nc = bacc.Bacc(None, target_bir_lowering=False)

    xT = nc.declare_dram_parameter("xT", [DIM, S], BF16, isOutput=False)
    wqk = nc.declare_dram_parameter("wqk", [DIM, 2 * HL * DH], BF16, isOutput=False)
    wv = nc.declare_dram_parameter("wv", [DIM, HL * DH], BF16, isOutput=False)
    cosq = nc.declare_dram_parameter("cosq", [128, S], BF16, isOutput=False)
    sinq = nc.declare_dram_parameter("sinq", [128, S], BF16, isOutput=False)
    wp = nc.declare_dram_parameter("wp", [DIM, DIM], BF16, isOutput=False)
    bp = nc.declare_dram_parameter("bp", [1, DIM], F32, isOutput=False)
    flags = nc.declare_dram_parameter("flags", [1, NB], mybir.dt.uint32,
                                      isOutput=False)
    out_d = nc.declare_dram_parameter("out", [SC, DIM], F32, isOutput=True)

    scale = DH ** -0.5

    with tile.TileContext(nc) as tc:
        with (
            tc.tile_pool(name="const", bufs=1) as const,
            tc.tile_pool(name="work", bufs=2) as work,
            tc.tile_pool(name="psum", bufs=2, space="PSUM") as psum,
            tc.tile_pool(name="dram", bufs=1, space="DRAM") as dram,
        ):
            # ---- input loads, spread over 4 non-ACT queues ----------------
            xT_sb = const.tile([128, KC, S], BF16)
            wqk_sb = const.tile([128, KC, 2 * HL * DH], BF16)
            wv_sb = const.tile([128, KC, HL * DH], BF16)
            wp_sb = const.tile([128, KC, DIM], BF16)
            cos_sb = const.tile([128, S], BF16)
            sin_sb = const.tile([128, S], BF16)
            bp_sb = const.tile([1, DIM], F32)

            # ---- per-block predication flags FIRST: tile_critical quiesces
            # prior queue activity, so it must precede every DMA issue
            with tc.tile_critical():
                conds = []
                for i in range(NB):
                    r = nc.gpsimd.alloc_register(f"flag_{i}")
                    nc.gpsimd.reg_load(r, flags[0:1, i:i + 1])
                    conds.append(nc.gpsimd.snap(r, donate=True, min_val=0,
                                                max_val=1))

            ones128 = const.tile([128, 128], F32)
            nc.vector.memset(ones128[:], 1.0)
            c4b = const.tile([128, 1], F32)
            nc.vector.memset(c4b[:], c4)

            # preload the exp table set while ScalarE is idle
            warm_sb = work.tile([1, 16], F32, tag="warm")
            nc.scalar.activation(warm_sb[:], ones128[0:1, 0:16], AF.Exp)

            # critical path: wqk + xT slab 0 (block-0 scores j=0..3), split
            # across the SP and ACT DGE queues (ACT is idle until ~11us)
            for k in range(3):
                nc.sync.dma_start(wqk_sb[:, k, :], wqk[k * 128:(k + 1) * 128, :])
            nc.scalar.dma_start(cos_sb[:, 0:1024], cosq[:, 0:1024])
            nc.scalar.dma_start(sin_sb[:, 0:1024], sinq[:, 0:1024])
            for k in range(3):
                nc.scalar.dma_start(wqk_sb[:, k + 3, :],
                                    wqk[(k + 3) * 128:(k + 4) * 128, :])
            for k in range(3):
                nc.sync.dma_start(xT_sb[:, k, 0:512],
                                  xT[k * 128:(k + 1) * 128, 0:512])
                nc.scalar.dma_start(xT_sb[:, k + 3, 0:512],
                                    xT[(k + 3) * 128:(k + 4) * 128, 0:512])

            # ---- collective warm-up (absorbs the barrier + cc init) -------
            # warm_in is fed by a cond-DMA that depends on the flags snap:
            # this forces the scheduler to place the gather AFTER the
            # tile_critical (whose entry quiesce would otherwise wait for
            # the gather, serializing the whole startup).
            warm_in = dram.tile([64, 16], BF16, tag="warm_in")
            warm_out = dram.tile([256, 16], BF16, tag="warm_out")
            warmsrc = const.tile([64, 16], BF16)
            nc.vector.memset(warmsrc[:], 0.0)
            nc.gpsimd.dma_start(warm_in[:], warmsrc[:], cond=conds[0])
            nc.gpsimd.collective_compute(
                "AllGather", mybir.AluOpType.bypass, replica_groups=GROUPS,
                ins=[warm_in[:].opt()], outs=[warm_out[:]],
            )

            # remaining xT slabs + tables + weights (non-critical)
            for sb in range(1, 4):
                sl = slice(sb * 512, (sb + 1) * 512)
                for k in range(KC):
                    q = nc.sync if k < 3 else nc.gpsimd
                    q.dma_start(xT_sb[:, k, sl], xT[k * 128:(k + 1) * 128, sl])
            nc.scalar.dma_start(cos_sb[:, 1024:2048], cosq[:, 1024:2048])
            nc.scalar.dma_start(sin_sb[:, 1024:2048], sinq[:, 1024:2048])
            for k in range(KC):
                nc.scalar.dma_start(wv_sb[:, k, :], wv[k * 128:(k + 1) * 128, :])
            nc.scalar.dma_start(bp_sb[:], bp[:])
            for k in range(KC):
                (nc.gpsimd if k % 2 else nc.sync).dma_start(
                    wp_sb[:, k, :], wp[k * 128:(k + 1) * 128, :])

            # ---- qk^T = wqk.T @ xT with fused RoPE ------------------------
            # wqk column order [q0, q1 | k0, k1 | q2, k2], channels
            # deinterleaved per head so rotate_half is a 32-partition swap.
            qkb = const.tile([128, 3, S], BF16)
            # head-2 partition-shift copies: qk2d = [k2 (lo) | q2 (hi)]
            qk2d = const.tile([128, S], BF16)

            def emit_qk_tile(mb, sb, dve_all=False):
                sl = slice(sb * 512, (sb + 1) * 512)
                ps = psum.tile([128, 512], F32, tag="ps_mm", bufs=1)
                for k in range(KC):
                    nc.tensor.matmul(
                        ps[:],
                        wqk_sb[:, k, mb * 128:(mb + 1) * 128],
                        xT_sb[:, k, sl],
                        start=(k == 0), stop=(k == KC - 1),
                    )
                qks = work.tile([128, 512], BF16, tag="qks", bufs=3)
                nc.vector.tensor_copy(qks[:], ps[:])
                rot = work.tile([128, 512], BF16, tag="rot", bufs=3)
                for g in range(2):
                    o = g * 64
                    nc.sync.dma_start(rot[o:o + 32, :], qks[o + 32:o + 64, :])
                    nc.sync.dma_start(rot[o + 32:o + 64, :], qks[o:o + 32, :])
                rots = work.tile([128, 512], BF16, tag="rots", bufs=2)
                nc.vector.tensor_mul(rots[:], rot[:], sin_sb[:, sl])
                tmp = work.tile([128, 512], BF16, tag="tmp", bufs=2)
                nc.vector.tensor_mul(tmp[:], qks[:], cos_sb[:, sl])
                nc.vector.tensor_add(qkb[:, mb, sl], tmp[:], rots[:])
                if mb == 2:
                    # per-slab head-2 swap, so sc2 never waits on all slabs
                    nc.sync.dma_start(qk2d[0:64, sl], qkb[64:128, 2, sl])
                    nc.sync.dma_start(qk2d[64:128, sl], qkb[0:64, 2, sl])

            # k01 slab 0 then q01 slab 0 unblocks scores j=0-3 early
            emit_qk_tile(1, 0, dve_all=True)
            emit_qk_tile(0, 0, dve_all=True)

            # v in [keys, ch]; slab per head = [v | ones | pad]; the memset-1
            # leaves pad columns at 1.0 (harmless extra denominator copies in
            # unread psum rows 65-127)
            v_aug = const.tile([128, NJ, HL * 128], BF16)
            for h in range(HL):
                nc.vector.memset(v_aug[:, :, h * 128 + DH:(h + 1) * 128], 1.0)

            def emit_v_chunk(st):
                ps = psum.tile([128, HL * DH], F32, tag="ps_mm", bufs=1)
                for k in range(KC):
                    nc.tensor.matmul(
                        ps[:],
                        xT_sb[:, k, st * 128:(st + 1) * 128],
                        wv_sb[:, k, :],
                        start=(k == 0), stop=(k == KC - 1),
                    )
                dst = v_aug[:, st, :].rearrange(
                    "p (h x) -> p h x", h=HL)[:, :, 0:DH]
                src = ps.rearrange("p (h x) -> p h x", h=HL)
                nc.vector.tensor_copy(dst, src)

            bp128 = const.tile([128, DIM], F32)

            def emit_bias_bcast():
                for o0, on in ((0, 512), (512, 256)):
                    psb = psum.tile([128, on], F32, tag="ps_mm", bufs=1)
                    nc.tensor.matmul(
                        psb[:], ones128[0:1, :], bp_sb[0:1, o0:o0 + on],
                        start=True, stop=True,
                    )
                    nc.vector.tensor_copy(bp128[:, o0:o0 + on], psb[:])

            # ---- attention ------------------------------------------------
            agZ1, agZ2 = [], []
            for b in range(NB):
                agZ1.append(dram.tile([512, SC], BF16, tag=f"agZ1_{b}",
                                      name=f"agZ1_{b}"))
                agZ2.append(dram.tile([256, SC], BF16, tag=f"agZ2_{b}",
                                      name=f"agZ2_{b}"))
            P01 = const.tile([128, NJ, 2, 512], BF16)
            P2 = [const.tile([128, NJ, 512], BF16, tag=f"P2_{i}", name=f"P2_{i}")
                  for i in range(2)]

            def emit_dve_exp(ps2, dst):
                ssb = work.tile([128, 2, 512], F32, tag="ssb", bufs=2)
                nc.vector.tensor_copy(ssb[:], ps2[:])
                hh = work.tile([128, 2, 512], F32, tag="hh", bufs=2)
                sflat = ssb[:].rearrange("p a b -> p (a b)")
                hflat = hh[:].rearrange("p a b -> p (a b)")
                nc.vector._custom_dve(
                    EXP1, out=hflat, in0=sflat, in1=c4b[:],
                    s0=c7, s1=c6, imm2=c5,
                )
                nc.vector._custom_dve(
                    EXP2, out=dst, in0=sflat, in1=hflat,
                    s0=c3, s1=c2, imm2=c1,
                )

            def emit_norm(ps_o, dst_d, r0, tag):
                # denominator sits on psum partition 64; copy to SBUF first
                # (custom-DVE reciprocal must NOT read PSUM), broadcast 1/den
                # across partitions on GpSimd, multiply straight out of PSUM
                den = work.tile([1, 512], F32, tag="den")
                nc.vector.tensor_copy(den[:], ps_o[64:65, :])
                rcp = work.tile([1, 512], F32, tag="rcp")
                nc.vector.reciprocal_approx_fast(rcp[:], den[:])
                rcpb = work.tile([DH, 512], F32, tag="rcpb")
                nc.gpsimd.partition_broadcast(rcpb[:], rcp[:], channels=DH)
                ob = work.tile([DH, 512], BF16, tag="ob", bufs=3)
                nc.vector.tensor_mul(ob[:], ps_o[0:DH, :], rcpb[:])
                nc.sync.dma_start(dst_d[r0:r0 + DH, :], ob[:])

            # DVE head-2 pairs per block (tail chunks of the t loop)
            DP = [0, 3, 3, 2]
            ob1_ds = [dram.tile([128, SC], BF16, tag=f"ob1_{b}",
                                name=f"ob1_{b}") for b in range(NB)]
            ob2_ds = [dram.tile([DH, SC], BF16, tag=f"ob2_{b}",
                                name=f"ob2_{b}") for b in range(NB)]

            def emit_sc01(b, j):
                isl = slice(b * 512, (b + 1) * 512)
                ps2 = psum.tile([128, 2, 512], F32, tag="ps_s")
                nc.tensor.matmul(
                    ps2[:, 0, :],
                    qkb[0:64, 1, j * 128:(j + 1) * 128],
                    qkb[0:64, 0, isl], start=True, stop=True,
                    tile_position=(0, 0),
                )
                nc.tensor.matmul(
                    ps2[:, 1, :],
                    qkb[64:128, 1, j * 128:(j + 1) * 128],
                    qkb[64:128, 0, isl], start=True, stop=True,
                    tile_position=(64, 0),
                )
                nc.scalar.activation(
                    P01[:, j, :, :], ps2[:], AF.Exp, scale=scale
                )

            def emit_sc2(b, t):
                isl = slice(b * 512, (b + 1) * 512)
                j0, j1 = 2 * t, 2 * t + 1
                P2b = P2[b % 2]
                ps2 = psum.tile([128, 2, 512], F32, tag="ps_s")
                nc.tensor.matmul(
                    ps2[:, 0, :],
                    qk2d[0:64, j0 * 128:(j0 + 1) * 128],
                    qkb[0:64, 2, isl], start=True, stop=True,
                    tile_position=(0, 0),
                )
                nc.tensor.matmul(
                    ps2[:, 1, :],
                    qkb[64:128, 2, j1 * 128:(j1 + 1) * 128],
                    qk2d[64:128, isl], start=True, stop=True,
                    tile_position=(64, 0),
                )
                if t < NJ // 2 - DP[b]:
                    nc.scalar.activation(
                        P2b[:, j0:j0 + 2, :], ps2[:], AF.Exp, scale=scale
                    )
                else:
                    emit_dve_exp(ps2, P2b[:, j0:j0 + 2, :])

            def emit_pv2_chunks(b, jcs, ps_o2):
                for jc in jcs:
                    nc.tensor.matmul(
                        ps_o2[:],
                        v_aug[:, jc, 2 * 128:3 * 128],
                        P2[b % 2][:, jc, :],
                        start=(jc == 0), stop=(jc == NJ - 1),
                    )

            def emit_gather(src, dst):
                nc.gpsimd.collective_compute(
                    "AllGather", mybir.AluOpType.bypass, replica_groups=GROUPS,
                    ins=[src[:].opt()], outs=[dst[:]],
                )

            # Software pipeline: stageA(b) interleaves block b-1's head-2
            # scores (+ its pv2 chunks trailing 2 pairs behind) with block
            # b's heads01 scores j=0..7; stageB(b) runs j=8..15 with both
            # pv01 accumulations trailing 2 chunks/iter, finishes block b-1's
            # pv2 + gather2, then norms + gather1 for block b.  This keeps
            # ScalarE exp-fed continuously across block boundaries.
            ps_o2_prev = None
            for b in range(NB):
                # ---------------- stageA ----------------
                if b >= 1:
                    ps_o2_prev = psum.tile([128, 512], F32, tag="ps_p2", bufs=1)
                for t in range(8):
                    if b >= 1:
                        emit_sc2(b - 1, t)
                    emit_sc01(b, t)
                    if b >= 1 and t >= 2:
                        emit_pv2_chunks(b - 1, (2 * (t - 2), 2 * t - 3),
                                        ps_o2_prev)
                    if b == 0:
                        if t == 0:
                            emit_qk_tile(1, 1)
                        elif t == 1:
                            emit_bias_bcast()
                        elif t in (2, 3):
                            emit_v_chunk(2 * (t - 2))
                            emit_v_chunk(2 * t - 3)
                        elif t == 4:
                            emit_qk_tile(1, 2)
                        elif t in (5, 7):
                            emit_v_chunk(t - 1)
                            emit_v_chunk(t)
                        elif t == 6:
                            emit_qk_tile(1, 3)
                    elif b == 1 and t == 0:
                        emit_qk_tile(0, 2)
                    elif b == 2 and t == 0:
                        emit_qk_tile(0, 3)
                # ---------------- stageB ----------------
                ps_o0 = psum.tile([128, 512], F32, tag="ps_o")
                ps_o1 = psum.tile([128, 512], F32, tag="ps_o")
                for j in range(8, 16):
                    emit_sc01(b, j)
                    if b == 0:
                        if j in (8, 10, 12, 14):
                            emit_qk_tile(2, (j - 8) // 2)
                        elif j == 9:
                            emit_v_chunk(8)
                            emit_v_chunk(9)
                        elif j == 11:
                            emit_v_chunk(10)
                            emit_v_chunk(11)
                        elif j == 13:
                            emit_v_chunk(12)
                            emit_v_chunk(13)
                            emit_qk_tile(0, 1)
                        elif j == 15:
                            emit_v_chunk(14)
                            emit_v_chunk(15)
                    for jc in (2 * (j - 8), 2 * j - 15):
                        for h, pso in ((0, ps_o0), (1, ps_o1)):
                            nc.tensor.matmul(
                                pso[:],
                                v_aug[:, jc, h * 128:(h + 1) * 128],
                                P01[:, jc, h, :],
                                start=(jc == 0), stop=(jc == NJ - 1),
                            )
                    if b >= 1:
                        if j == 8:
                            emit_pv2_chunks(b - 1, (12, 13), ps_o2_prev)
                        elif j == 9:
                            emit_pv2_chunks(b - 1, (14, 15), ps_o2_prev)
                            emit_norm(ps_o2_prev, ob2_ds[b - 1], 0, f"{b-1}_2")
                            emit_gather(ob2_ds[b - 1], agZ2[b - 1])
                emit_norm(ps_o0, ob1_ds[b], 0, f"{b}_0")
                emit_norm(ps_o1, ob1_ds[b], DH, f"{b}_1")
                emit_gather(ob1_ds[b], agZ1[b])

            # epilogue: block 3 head-2 scores + pv2 + final small gather
            ps_o2_prev = psum.tile([128, 512], F32, tag="ps_p2", bufs=1)
            for t in range(8):
                emit_sc2(3, t)
                if t >= 2:
                    emit_pv2_chunks(3, (2 * (t - 2), 2 * t - 3), ps_o2_prev)
            emit_pv2_chunks(3, (12, 13, 14, 15), ps_o2_prev)
            emit_norm(ps_o2_prev, ob2_ds[3], 0, "3_2")
            emit_gather(ob2_ds[3], agZ2[3])

            # keep the PE's HAM window busy while the last gather lands
            for w in range(8):
                wps = psum.tile([128, 512], F32, tag="ps_p2", bufs=1)
                nc.tensor.matmul(
                    wps[:], qkb[:, 0, 0:128], qkb[:, 1, 0:512],
                    start=True, stop=True,
                )

            # ---- output projection on my 512-row slice --------------------
            # cond-predicated loads: only block g's flag is 1 on core g
            ag1_sb = const.tile([128, 4, SC], BF16)
            ag2_sb = const.tile([128, 2, SC], BF16)
            for b in range(NB):
                nc.gpsimd.dma_start(
                    ag1_sb[:],
                    agZ1[b][:].rearrange("(k p) n -> p k n", p=128),
                    cond=conds[b],
                )
            for b in range(NB):
                nc.gpsimd.dma_start(
                    ag2_sb[:],
                    agZ2[b][:].rearrange("(k p) n -> p k n", p=128),
                    cond=conds[b],
                )

            for m in range(SC // 128):
                for o0, on in ((0, 512), (512, 256)):
                    ps_p = psum.tile([128, on], F32, tag="ps_o")
                    for k in range(4):
                        nc.tensor.matmul(
                            ps_p[:],
                            ag1_sb[:, k, m * 128:(m + 1) * 128],
                            wp_sb[:, k, o0:o0 + on],
                            start=(k == 0), stop=False,
                        )
                    for k2 in range(2):
                        nc.tensor.matmul(
                            ps_p[:],
                            ag2_sb[:, k2, m * 128:(m + 1) * 128],
                            wp_sb[:, 4 + k2, o0:o0 + on],
                            start=False, stop=(k2 == 1),
                        )
                    po = work.tile([128, on], F32, tag="po", bufs=4)
                    nc.vector.tensor_add(po[:], ps_p[:], bp128[:, o0:o0 + on])
                    (nc.sync if m % 2 else nc.gpsimd).dma_start(
                        out_d[m * 128:(m + 1) * 128, o0:o0 + on], po[:]
                    )

    nc.compile()
    return nc


def _rope_tables():
    bf16 = ml_dtypes.bfloat16
    inv = (1.0 / (THETA ** (np.arange(0, DH, 2, dtype=np.float32) / DH))).astype(
        np.float32
    )
    pos = np.arange(S, dtype=np.float32)
    f = pos[:, None] * inv[None, :]           # [S, 32] f32, matches reference
    c = np.cos(f).T.astype(np.float32)        # [32, S]
    s = np.sin(f).T.astype(np.float32)
    cos64 = np.concatenate([c, c], axis=0)    # rows i and 32+i = cos(f_i)
    sin64 = np.concatenate([-s, s], axis=0)   # sign folded for rotate_half
    return (
        np.concatenate([cos64, cos64], axis=0).astype(bf16),  # [128, S]
        np.concatenate([sin64, sin64], axis=0).astype(bf16),
    )


def _shard_inputs(x, W_qkv, W_proj, b_proj):
    bf16 = ml_dtypes.bfloat16
    cos128, sin128 = _rope_tables()
    # deinterleave perm: new[i] = orig[2i] (i<32), new[32+i] = orig[2i+1]
    perm = np.concatenate([np.arange(0, DH, 2), np.arange(1, DH, 2)])
    # W_proj input-channel order after the split gathers:
    #   rows 0..511:  group g, heads {0,1}:  idx 128g + 64h + d
    #   rows 512..767: group g, head 2:      idx 512 + 64g + d
    ch = np.empty(DIM, dtype=np.int64)
    for g in range(4):
        for h in range(2):
            ch[128 * g + 64 * h: 128 * g + 64 * h + 64] = \
                (HL * g + h) * DH + np.arange(DH)
        ch[512 + 64 * g: 512 + 64 * g + 64] = (HL * g + 2) * DH + np.arange(DH)
    wp_t = np.ascontiguousarray(W_proj.T[ch]).astype(bf16)      # [c_perm, o]
    bp_r = np.ascontiguousarray(b_proj[None, :]).astype(np.float32)
    in_maps = []
    for c in range(N_CORES):
        b, g = c // 4, c % 4
        hs = [HL * g + i for i in range(HL)]
        q_r = [h * DH + perm for h in hs]
        k_r = [DIM + h * DH + perm for h in hs]
        # column order [q0, q1 | k0, k1 | q2, k2] to align base partitions
        qk_rows = np.concatenate([q_r[0], q_r[1], k_r[0], k_r[1], q_r[2], k_r[2]])
        v_rows = np.concatenate([2 * DIM + h * DH + np.arange(DH) for h in hs])
        flag = np.zeros(NB, dtype=np.uint32)
        flag[g] = 1
        in_maps.append({
            "xT": np.ascontiguousarray(x[b].T).astype(bf16),
            "wqk": np.ascontiguousarray(W_qkv[qk_rows].T).astype(bf16),
            "wv": np.ascontiguousarray(W_qkv[v_rows].T).astype(bf16),
            "cosq": cos128,
            "sinq": sin128,
            "wp": wp_t,
            "bp": bp_r,
            "flags": flag[None, :],
        })
    return in_maps


def run(inputs, trace=False, tmpdir=None):
    if "nc" not in _CACHED:
        _CACHED["nc"] = _build()
    nc = _CACHED["nc"]
    in_maps = _shard_inputs(
        inputs["x"], inputs["W_qkv"], inputs["W_proj"], inputs["b_proj"]
    )
    res = bass_utils.run_bass_kernel_spmd(
        nc, in_maps, core_ids=list(range(N_CORES)), trace=trace, tmpdir=tmpdir
    )
    out = np.empty((B, S, DIM), dtype=np.float32)
    for c in range(N_CORES):
        b, g = c // 4, c % 4
        out[b, g * SC:(g + 1) * SC, :] = res.results[c]["out"]
    return out, res


def kernel(**inputs):
    out, _ = run(inputs, trace=False)
    return out


# revision 16
# speedup vs baseline: 1.4599x; 1.0752x over previous
"""Distributed multi-head attention (RoPE, non-causal) for 8 TRN2 NeuronCores.

Problem: B=2, S=2048, DIM=768, H=12, HEAD_DIM=64, f32 I/O.

Sharding: 24 (batch, head) pairs -> core c handles batch c//4 and heads
3*(c%4) .. 3*(c%4)+2.  Per core (bf16 matmuls, f32 PSUM):
  * The softmax exp is the #1 engine load (12.6M elems/core, ScalarE-only
    at 1 elem/cyc/partition).  It is split: heads 0/1 + the first head-2
    chunks go to ScalarE ACT-exp; the tail head-2 chunks go to the DVE via
    two chained custom-DVE ops evaluating a degree-7 polynomial
    p(x) ~= exp(x/8) on |x|<=24 (raw-score range is +-21.4; poly rel err
    1.2e-3, far under the bf16 noise floor).  Custom-DVE ops cannot read
    PSUM, so each DVE tile is staged by one DVE tensor_copy.
  * QKV projection emitted K-first, slab-by-slab, so scores j=0..3 and the
    first exp fire ~10us earlier than a bulk-load design; input DMAs are
    issued across 4 engine queues (none on ScalarE, which is the critical
    engine).
  * RoPE fused out of PSUM in bf16 (2-4x DVE modes): DVE copies PSUM->SBUF
    bf16, rotate_half = 32-row partition-swap DMA (sign folded in the sin
    table), sin-mul on DVE, cos-mul + add on GpSimd (SBUF-only engine).
  * scores: heads 0,1 on PE row-halves (tile_position (0,0)/(64,0));
    head 2 pairs even/odd key chunks via 2 partition-shift copies.
  * out^T accumulated via lhsT=[v | ones | pad] (M=128); softmax
    denominator lands on psum partition 64; per head: DVE den copy +
    reciprocal_approx_fast + GpSimd partition_broadcast + one DVE multiply
    straight out of PSUM.
  * Per 512-query block TWO AllGathers: heads01 [128,512] as soon as both
    finish, head2 [64,512] after -- so the kernel tail only waits on a
    65KB collective.  A tiny warm-up AllGather is issued at t~0 to absorb
    the one-time ~12us collective-engine init.  W_proj rows are permuted
    host-side to match the gathered [heads01-of-all-groups | head2s]
    channel order.
  * Output projection on the core's own 512-row slice via cond-predicated
    loads (identical SPMD program, flags input picks the block).
Host side only shards/permutes/casts inputs and concatenates the 8
output slices.
"""

import sys

sys.path.insert(0, "/opt/trn_rl_repo")

import numpy as np
import ml_dtypes

import concourse.bass as bass
import concourse.mybir as mybir
import concourse.tile as tile
from concourse import bacc, bass_utils

BF16 = mybir.dt.bfloat16
F32 = mybir.dt.float32
AF = mybir.ActivationFunctionType

B, S, DIM, H, DH = 2, 2048, 768, 12, 64
THETA = 10000.0
N_CORES = 8
GROUPS = [[0, 1, 2, 3], [4, 5, 6, 7]]
HL = 3           # heads per core
SC = S // 4      # per-core output row slice (512)
KC = DIM // 128  # 6 contraction chunks
NJ = S // 128    # 16 key chunks
NB = 4           # query blocks per core batch

# head-2 exp chunk-pairs per block routed to the DVE poly-exp (0..8)
DVE_PAIRS = 4
DVE_PSUM_DIRECT = True

_CACHED = {}

# --------------------------------------------------------------------------
# Custom DVE exp(x/8): degree-7 poly, p(0)=1, fitted minimax-relative on
# |x|<=24 (raw scores span +-21.4).  Split into two 7-stage ops:
#   PART1: H = (((c7*x + c6)*x + c5)*x + c4)*x        (c4 spilled via in1)
#   PART2: out = ((((H + c3)*x + c2)*x + c1)*x) + 1
_EXP_C = [
    1.25124039e-01, 7.79104663e-03, 3.19739592e-04, 1.00846613e-05,
    2.82333267e-07, 6.39666992e-09, 7.30440201e-11,
]


def _register_exp_ops():
    if "ops" in _CACHED:
        return _CACHED["ops"]
    import concourse.dve_ops as dve_ops
    from concourse.dve_ops import DveOp
    from concourse.dve_spec import C0, C1, C2, C3, One, Spec, Src0, Src1, \
        _spill_c3_to_src1, lower
    from concourse.dve_uop import DveOpSpec

    def ref1(in0, in1, c0, c1, c2):
        x = in0.astype(np.float32)
        return ((((c0 * x + c1) * x + c2) * x + in1) * x).astype(np.float32)

    def ref2(in0, in1, c0, c1, c2):
        x = in0.astype(np.float32)
        h = in1.astype(np.float32)
        return (((h + c0) * x + c1) * x + c2) * x + np.float32(1.0)

    body1 = _spill_c3_to_src1((((Src0 * C0 + C1) * Src0 + C2) * Src0 + C3) * Src0)
    body2 = (((Src1 + C0) * Src0 + C1) * Src0 + C2) * Src0 + One

    def mk(name, body, ref):
        if name in dve_ops._SUB_OPCODE_FOR_NAME:
            return next(o for o in dve_ops.OPS if o.name == name)
        spec = Spec(body=body, reference=ref)
        shas = {}
        for ver in ("v3", "v4"):
            shas[ver] = DveOpSpec(
                name=name, opcode=31, uops=lower(spec, ver=ver), rd1_en=True
            ).sha(ver)
        op = DveOp(name, spec, subdim=False, uops_sha=shas)
        dve_ops.OPS.append(op)
        dve_ops.CUSTOM_DVE_SPECS[name] = spec
        dve_ops._SUB_OPCODE_FOR_NAME[name] = (
            max(dve_ops._SUB_OPCODE_FOR_NAME.values()) + 1
        )
        assert max(dve_ops._SUB_OPCODE_FOR_NAME.values()) < 0x20
        return op

    e1 = mk("EXP8_PART1_ANT", body1, ref1)
    e2 = mk("EXP8_PART2_ANT", body2, ref2)
    _CACHED["ops"] = (e1, e2)
    return e1, e2


def _build():
    """Build the SPMD Bacc graph (identical on all 8 cores)."""
    EXP1, EXP2 = _register_exp_ops()
    c1, c2, c3, c4, c5, c6, c7 = [float(np.float32(v)) for v in _EXP_C]

    nc = bacc.Bacc(None, target_bir_lowering=False)

    xT = nc.declare_dram_parameter("xT", [DIM, S], BF16, isOutput=False)
    wqk = nc.declare_dram_parameter("wqk", [DIM, 2 * HL * DH], BF16, isOutput=False)
    wv = nc.declare_dram_parameter("wv", [DIM, HL * DH], BF16, isOutput=False)
    cosq = nc.declare_dram_parameter("cosq", [128, S], BF16, isOutput=False)
    sinq = nc.declare_dram_parameter("sinq", [128, S], BF16, isOutput=False)
    wp = nc.declare_dram_parameter("wp", [DIM, DIM], BF16, isOutput=False)
    bp = nc.declare_dram_parameter("bp", [1, DIM], F32, isOutput=False)
    flags = nc.declare_dram_parameter("flags", [1, NB], mybir.dt.uint32,
                                      isOutput=False)
    out_d = nc.declare_dram_parameter("out", [SC, DIM], F32, isOutput=True)

    scale = DH ** -0.5

    with tile.TileContext(nc) as tc:
        with (
            tc.tile_pool(name="const", bufs=1) as const,
            tc.tile_pool(name="work", bufs=2) as work,
            tc.tile_pool(name="psum", bufs=2, space="PSUM") as psum,
            tc.tile_pool(name="dram", bufs=1, space="DRAM") as dram,
        ):
            # ---- input loads, spread over 4 non-ACT queues ----------------
            xT_sb = const.tile([128, KC, S], BF16)
            wqk_sb = const.tile([128, KC, 2 * HL * DH], BF16)
            wv_sb = const.tile([128, KC, HL * DH], BF16)
            wp_sb = const.tile([128, KC, DIM], BF16)
            cos_sb = const.tile([128, S], BF16)
            sin_sb = const.tile([128, S], BF16)
            bp_sb = const.tile([1, DIM], F32)

            # ---- per-block predication flags FIRST: tile_critical quiesces
            # prior queue activity, so it must precede every DMA issue
            with tc.tile_critical():
                conds = []
                for i in range(NB):
                    r = nc.gpsimd.alloc_register(f"flag_{i}")
                    nc.gpsimd.reg_load(r, flags[0:1, i:i + 1])
                    conds.append(nc.gpsimd.snap(r, donate=True, min_val=0,
                                                max_val=1))

            ones128 = const.tile([128, 128], F32)
            nc.vector.memset(ones128[:], 1.0)
            c4b = const.tile([128, 1], F32)
            nc.vector.memset(c4b[:], c4)

            # preload the exp table set while ScalarE is idle
            warm_sb = work.tile([1, 16], F32, tag="warm")
            nc.scalar.activation(warm_sb[:], ones128[0:1, 0:16], AF.Exp)

            # critical path: wqk + xT slab 0 (block-0 scores j=0..3), split
            # across the SP and ACT DGE queues (ACT is idle until ~11us)
            for k in range(3):
                nc.sync.dma_start(wqk_sb[:, k, :], wqk[k * 128:(k + 1) * 128, :])
            nc.scalar.dma_start(cos_sb[:, 0:1024], cosq[:, 0:1024])
            nc.scalar.dma_start(sin_sb[:, 0:1024], sinq[:, 0:1024])
            for k in range(3):
                nc.scalar.dma_start(wqk_sb[:, k + 3, :],
                                    wqk[(k + 3) * 128:(k + 4) * 128, :])
            for k in range(3):
                nc.sync.dma_start(xT_sb[:, k, 0:512],
                                  xT[k * 128:(k + 1) * 128, 0:512])
                nc.scalar.dma_start(xT_sb[:, k + 3, 0:512],
                                    xT[(k + 3) * 128:(k + 4) * 128, 0:512])

            # ---- collective warm-up (absorbs the barrier + cc init) -------
            # warm_in is fed by a cond-DMA that depends on the flags snap:
            # this forces the scheduler to place the gather AFTER the
            # tile_critical (whose entry quiesce would otherwise wait for
            # the gather, serializing the whole startup).
            warm_in = dram.tile([64, 16], BF16, tag="warm_in")
            warm_out = dram.tile([256, 16], BF16, tag="warm_out")
            warmsrc = const.tile([64, 16], BF16)
            nc.vector.memset(warmsrc[:], 0.0)
            nc.gpsimd.dma_start(warm_in[:], warmsrc[:], cond=conds[0])
            nc.gpsimd.collective_compute(
                "AllGather", mybir.AluOpType.bypass, replica_groups=GROUPS,
                ins=[warm_in[:].opt()], outs=[warm_out[:]],
            )

            # remaining xT slabs + tables + weights (non-critical)
            for sb in range(1, 4):
                sl = slice(sb * 512, (sb + 1) * 512)
                for k in range(KC):
                    q = nc.sync if k < 3 else nc.gpsimd
                    q.dma_start(xT_sb[:, k, sl], xT[k * 128:(k + 1) * 128, sl])
            nc.scalar.dma_start(cos_sb[:, 1024:2048], cosq[:, 1024:2048])
            nc.scalar.dma_start(sin_sb[:, 1024:2048], sinq[:, 1024:2048])
            for k in range(KC):
                nc.scalar.dma_start(wv_sb[:, k, :], wv[k * 128:(k + 1) * 128, :])
            nc.scalar.dma_start(bp_sb[:], bp[:])
            for k in range(KC):
                (nc.gpsimd if k % 2 else nc.sync).dma_start(
                    wp_sb[:, k, :], wp[k * 128:(k + 1) * 128, :])

            # ---- qk^T = wqk.T @ xT with fused RoPE ------------------------
            # wqk column order [q0, q1 | k0, k1 | q2, k2], channels
            # deinterleaved per head so rotate_half is a 32-partition swap.
            qkb = const.tile([128, 3, S], BF16)
            # head-2 partition-shift copies: qk2d = [k2 (lo) | q2 (hi)]
            qk2d = const.tile([128, S], BF16)

            def emit_qk_tile(mb, sb, dve_all=False):
                sl = slice(sb * 512, (sb + 1) * 512)
                ps = psum.tile([128, 512], F32, tag="ps_mm", bufs=1)
                for k in range(KC):
                    nc.tensor.matmul(
                        ps[:],
                        wqk_sb[:, k, mb * 128:(mb + 1) * 128],
                        xT_sb[:, k, sl],
                        start=(k == 0), stop=(k == KC - 1),
                    )
                qks = work.tile([128, 512], BF16, tag="qks", bufs=3)
                nc.vector.tensor_copy(qks[:], ps[:])
                rot = work.tile([128, 512], BF16, tag="rot", bufs=3)
                for g in range(2):
                    o = g * 64
                    nc.sync.dma_start(rot[o:o + 32, :], qks[o + 32:o + 64, :])
                    nc.sync.dma_start(rot[o + 32:o + 64, :], qks[o:o + 32, :])
                rots = work.tile([128, 512], BF16, tag="rots", bufs=2)
                nc.vector.tensor_mul(rots[:], rot[:], sin_sb[:, sl])
                tmp = work.tile([128, 512], BF16, tag="tmp", bufs=2)
                nc.vector.tensor_mul(tmp[:], qks[:], cos_sb[:, sl])
                nc.vector.tensor_add(qkb[:, mb, sl], tmp[:], rots[:])
                if mb == 2:
                    # per-slab head-2 swap, so sc2 never waits on all slabs
                    nc.sync.dma_start(qk2d[0:64, sl], qkb[64:128, 2, sl])
                    nc.sync.dma_start(qk2d[64:128, sl], qkb[0:64, 2, sl])

            # k01 slab 0 then q01 slab 0 unblocks scores j=0-3 early
            emit_qk_tile(1, 0, dve_all=True)
            emit_qk_tile(0, 0, dve_all=True)

            # v in [keys, ch]; slab per head = [v | ones | pad]; the memset-1
            # leaves pad columns at 1.0 (harmless extra denominator copies in
            # unread psum rows 65-127)
            v_aug = const.tile([128, NJ, HL * 128], BF16)
            for h in range(HL):
                nc.vector.memset(v_aug[:, :, h * 128 + DH:(h + 1) * 128], 1.0)

            def emit_v_chunk(st):
                ps = psum.tile([128, HL * DH], F32, tag="ps_mm", bufs=1)
                for k in range(KC):
                    nc.tensor.matmul(
                        ps[:],
                        xT_sb[:, k, st * 128:(st + 1) * 128],
                        wv_sb[:, k, :],
                        start=(k == 0), stop=(k == KC - 1),
                    )
                dst = v_aug[:, st, :].rearrange(
                    "p (h x) -> p h x", h=HL)[:, :, 0:DH]
                src = ps.rearrange("p (h x) -> p h x", h=HL)
                nc.vector.tensor_copy(dst, src)

            bp128 = const.tile([128, DIM], F32)

            def emit_bias_bcast():
                for o0, on in ((0, 512), (512, 256)):
                    psb = psum.tile([128, on], F32, tag="ps_mm", bufs=1)
                    nc.tensor.matmul(
                        psb[:], ones128[0:1, :], bp_sb[0:1, o0:o0 + on],
                        start=True, stop=True,
                    )
                    nc.vector.tensor_copy(bp128[:, o0:o0 + on], psb[:])

            # ---- attention ------------------------------------------------
            agZ = [dram.tile([DIM, SC], BF16, tag=f"agZ_{b}",
                             name=f"agZ_{b}") for b in range(NB)]
            P01 = const.tile([128, NJ, 2, 512], BF16)
            P2 = [const.tile([128, NJ, 512], BF16, tag=f"P2_{i}", name=f"P2_{i}")
                  for i in range(2)]

            def emit_dve_exp(ps2, dst):
                hh = work.tile([128, 2, 512], F32, tag="hh", bufs=2)
                hflat = hh[:].rearrange("p a b -> p (a b)")
                if DVE_PSUM_DIRECT:
                    sflat = ps2[:].rearrange("p a b -> p (a b)")
                else:
                    ssb = work.tile([128, 2, 512], F32, tag="ssb", bufs=2)
                    nc.vector.tensor_copy(ssb[:], ps2[:])
                    sflat = ssb[:].rearrange("p a b -> p (a b)")
                nc.vector._custom_dve(
                    EXP1, out=hflat, in0=sflat, in1=c4b[:],
                    s0=c7, s1=c6, imm2=c5,
                )
                nc.vector._custom_dve(
                    EXP2, out=dst, in0=sflat, in1=hflat,
                    s0=c3, s1=c2, imm2=c1,
                )

            def emit_norm(ps_o, dst_d, r0, tag):
                # denominator sits on psum partition 64; copy to SBUF first
                # (custom-DVE reciprocal must NOT read PSUM), broadcast 1/den
                # across partitions on GpSimd, multiply straight out of PSUM
                den = work.tile([1, 512], F32, tag="den")
                nc.vector.tensor_copy(den[:], ps_o[64:65, :])
                rcp = work.tile([1, 512], F32, tag="rcp")
                nc.vector.reciprocal_approx_fast(rcp[:], den[:])
                rcpb = work.tile([DH, 512], F32, tag="rcpb")
                nc.gpsimd.partition_broadcast(rcpb[:], rcp[:], channels=DH)
                ob = work.tile([DH, 512], BF16, tag="ob", bufs=3)
                nc.vector.tensor_mul(ob[:], ps_o[0:DH, :], rcpb[:])
                nc.sync.dma_start(dst_d[r0:r0 + DH, :], ob[:])

            # DVE head-2 pairs per block (tail chunks of the t loop)
            DP = [2, 4, 4, 3]
            ob_ds = [dram.tile([HL * DH, SC], BF16, tag=f"ob_{b}",
                               name=f"ob_{b}") for b in range(NB)]

            def emit_sc01(b, j):
                isl = slice(b * 512, (b + 1) * 512)
                ps2 = psum.tile([128, 2, 512], F32, tag="ps_s")
                nc.tensor.matmul(
                    ps2[:, 0, :],
                    qkb[0:64, 1, j * 128:(j + 1) * 128],
                    qkb[0:64, 0, isl], start=True, stop=True,
                    tile_position=(0, 0),
                )
                nc.tensor.matmul(
                    ps2[:, 1, :],
                    qkb[64:128, 1, j * 128:(j + 1) * 128],
                    qkb[64:128, 0, isl], start=True, stop=True,
                    tile_position=(64, 0),
                )
                nc.scalar.activation(
                    P01[:, j, :, :], ps2[:], AF.Exp, scale=scale
                )

            def emit_sc2(b, t):
                isl = slice(b * 512, (b + 1) * 512)
                j0, j1 = 2 * t, 2 * t + 1
                P2b = P2[b % 2]
                ps2 = psum.tile([128, 2, 512], F32, tag="ps_s")
                nc.tensor.matmul(
                    ps2[:, 0, :],
                    qk2d[0:64, j0 * 128:(j0 + 1) * 128],
                    qkb[0:64, 2, isl], start=True, stop=True,
                    tile_position=(0, 0),
                )
                nc.tensor.matmul(
                    ps2[:, 1, :],
                    qkb[64:128, 2, j1 * 128:(j1 + 1) * 128],
                    qk2d[64:128, isl], start=True, stop=True,
                    tile_position=(64, 0),
                )
                if t < NJ // 2 - DP[b]:
                    nc.scalar.activation(
                        P2b[:, j0:j0 + 2, :], ps2[:], AF.Exp, scale=scale
                    )
                else:
                    emit_dve_exp(ps2, P2b[:, j0:j0 + 2, :])

            def emit_pv2_chunks(b, jcs, ps_o2):
                for jc in jcs:
                    nc.tensor.matmul(
                        ps_o2[:],
                        v_aug[:, jc, 2 * 128:3 * 128],
                        P2[b % 2][:, jc, :],
                        start=(jc == 0), stop=(jc == NJ - 1),
                    )

            def emit_gather(src, dst):
                nc.gpsimd.collective_compute(
                    "AllGather", mybir.AluOpType.bypass, replica_groups=GROUPS,
                    ins=[src[:].opt()], outs=[dst[:]],
                )

            # Software pipeline: stageA(b) interleaves block b-1's head-2
            # scores (+ its pv2 chunks trailing 2 pairs behind) with block
            # b's heads01 scores j=0..7; stageB(b) runs j=8..15 with both
            # pv01 accumulations trailing 2 chunks/iter, finishes block b-1's
            # pv2 + gather2, then norms + gather1 for block b.  This keeps
            # ScalarE exp-fed continuously across block boundaries.
            ps_o2_prev = None
            for b in range(NB):
                # ---------------- stageA ----------------
                if b >= 1:
                    ps_o2_prev = psum.tile([128, 512], F32, tag="ps_p2", bufs=1)
                for t in range(8):
                    if b >= 2 or (b == 1 and t >= 6):
                        emit_sc2(b - 1, t)
                    emit_sc01(b, t)
                    if b >= 1 and t >= 2:
                        emit_pv2_chunks(b - 1, (2 * (t - 2), 2 * t - 3),
                                        ps_o2_prev)
                    if b == 0:
                        if t == 0:
                            emit_qk_tile(1, 1)
                        elif t == 1:
                            emit_bias_bcast()
                        elif t in (2, 3):
                            emit_v_chunk(2 * (t - 2))
                            emit_v_chunk(2 * t - 3)
                        elif t == 4:
                            emit_qk_tile(1, 2)
                        elif t in (5, 7):
                            emit_v_chunk(t - 1)
                            emit_v_chunk(t)
                        elif t == 6:
                            emit_qk_tile(1, 3)
                    elif b == 1 and t == 0:
                        emit_qk_tile(0, 2)
                    elif b == 2 and t == 0:
                        emit_qk_tile(0, 3)
                # ---------------- stageB ----------------
                ps_o0 = psum.tile([128, 512], F32, tag="ps_o")
                ps_o1 = psum.tile([128, 512], F32, tag="ps_o")
                for j in range(8, 16):
                    emit_sc01(b, j)
                    if b == 0:
                        if j in (8, 10, 12, 14):
                            emit_qk_tile(2, (j - 8) // 2)
                        elif j == 9:
                            emit_v_chunk(8)
                            emit_v_chunk(9)
                        elif j == 11:
                            emit_v_chunk(10)
                            emit_v_chunk(11)
                        elif j == 13:
                            emit_v_chunk(12)
                            emit_v_chunk(13)
                            emit_qk_tile(0, 1)
                        elif j == 15:
                            emit_v_chunk(14)
                            emit_v_chunk(15)
                    for jc in (2 * (j - 8), 2 * j - 15):
                        for h, pso in ((0, ps_o0), (1, ps_o1)):
                            nc.tensor.matmul(
                                pso[:],
                                v_aug[:, jc, h * 128:(h + 1) * 128],
                                P01[:, jc, h, :],
                                start=(jc == 0), stop=(jc == NJ - 1),
                            )
                    if b >= 1:
                        if j == 8:
                            emit_pv2_chunks(b - 1, (12, 13), ps_o2_prev)
                        elif j == 9:
                            emit_pv2_chunks(b - 1, (14, 15), ps_o2_prev)
                            emit_norm(ps_o2_prev, ob_ds[b - 1], 2 * DH,
                                      f"{b-1}_2")
                            emit_gather(ob_ds[b - 1], agZ[b - 1])
                    if b == 0 and j >= 10:
                        emit_sc2(0, j - 10)
                emit_norm(ps_o0, ob_ds[b], 0, f"{b}_0")
                emit_norm(ps_o1, ob_ds[b], DH, f"{b}_1")

            # epilogue: block 3 head-2 scores + pv2 + final small gather
            ps_o2_prev = psum.tile([128, 512], F32, tag="ps_p2", bufs=1)
            for t in range(8):
                emit_sc2(3, t)
                if t >= 2:
                    emit_pv2_chunks(3, (2 * (t - 2), 2 * t - 3), ps_o2_prev)
            emit_pv2_chunks(3, (12, 13, 14, 15), ps_o2_prev)
            emit_norm(ps_o2_prev, ob_ds[3], 2 * DH, "3_2")
            emit_gather(ob_ds[3], agZ[3])

            # keep the PE's HAM window busy while the last gather lands
            for w in range(8):
                wps = psum.tile([128, 512], F32, tag="ps_p2", bufs=1)
                nc.tensor.matmul(
                    wps[:], qkb[:, 0, 0:128], qkb[:, 1, 0:512],
                    start=True, stop=True,
                )

            # ---- output projection on my 512-row slice --------------------
            # cond-predicated loads: only block g's flag is 1 on core g
            ag_sb = const.tile([128, KC, SC], BF16)
            for b in range(NB):
                nc.gpsimd.dma_start(
                    ag_sb[:],
                    agZ[b][:].rearrange("(k p) n -> p k n", p=128),
                    cond=conds[b],
                )

            for m in range(SC // 128):
                for o0, on in ((0, 512), (512, 256)):
                    ps_p = psum.tile([128, on], F32, tag="ps_o")
                    for k in range(KC):
                        nc.tensor.matmul(
                            ps_p[:],
                            ag_sb[:, k, m * 128:(m + 1) * 128],
                            wp_sb[:, k, o0:o0 + on],
                            start=(k == 0), stop=(k == KC - 1),
                        )
                    po = work.tile([128, on], F32, tag="po", bufs=4)
                    nc.vector.tensor_add(po[:], ps_p[:], bp128[:, o0:o0 + on])
                    (nc.sync if m % 2 else nc.gpsimd).dma_start(
                        out_d[m * 128:(m + 1) * 128, o0:o0 + on], po[:]
                    )

    nc.compile()
    return nc


def _rope_tables():
    bf16 = ml_dtypes.bfloat16
    inv = (1.0 / (THETA ** (np.arange(0, DH, 2, dtype=np.float32) / DH))).astype(
        np.float32
    )
    pos = np.arange(S, dtype=np.float32)
    f = pos[:, None] * inv[None, :]           # [S, 32] f32, matches reference
    c = np.cos(f).T.astype(np.float32)        # [32, S]
    s = np.sin(f).T.astype(np.float32)
    cos64 = np.concatenate([c, c], axis=0)    # rows i and 32+i = cos(f_i)
    sin64 = np.concatenate([-s, s], axis=0)   # sign folded for rotate_half
    return (
        np.concatenate([cos64, cos64], axis=0).astype(bf16),  # [128, S]
        np.concatenate([sin64, sin64], axis=0).astype(bf16),
    )


def _shard_inputs(x, W_qkv, W_proj, b_proj):
    bf16 = ml_dtypes.bfloat16
    cos128, sin128 = _rope_tables()
    # deinterleave perm: new[i] = orig[2i] (i<32), new[32+i] = orig[2i+1]
    perm = np.concatenate([np.arange(0, DH, 2), np.arange(1, DH, 2)])
    wp_t = np.ascontiguousarray(W_proj.T).astype(bf16)          # [c, o]
    bp_r = np.ascontiguousarray(b_proj[None, :]).astype(np.float32)
    in_maps = []
    for c in range(N_CORES):
        b, g = c // 4, c % 4
        hs = [HL * g + i for i in range(HL)]
        q_r = [h * DH + perm for h in hs]
        k_r = [DIM + h * DH + perm for h in hs]
        # column order [q0, q1 | k0, k1 | q2, k2] to align base partitions
        qk_rows = np.concatenate([q_r[0], q_r[1], k_r[0], k_r[1], q_r[2], k_r[2]])
        v_rows = np.concatenate([2 * DIM + h * DH + np.arange(DH) for h in hs])
        flag = np.zeros(NB, dtype=np.uint32)
        flag[g] = 1
        in_maps.append({
            "xT": np.ascontiguousarray(x[b].T).astype(bf16),
            "wqk": np.ascontiguousarray(W_qkv[qk_rows].T).astype(bf16),
            "wv": np.ascontiguousarray(W_qkv[v_rows].T).astype(bf16),
            "cosq": cos128,
            "sinq": sin128,
            "wp": wp_t,
            "bp": bp_r,
            "flags": flag[None, :],
        })
    return in_maps


def run(inputs, trace=False, tmpdir=None):
    if "nc" not in _CACHED:
        _CACHED["nc"] = _build()
    nc = _CACHED["nc"]
    in_maps = _shard_inputs(
        inputs["x"], inputs["W_qkv"], inputs["W_proj"], inputs["b_proj"]
    )
    res = bass_utils.run_bass_kernel_spmd(
        nc, in_maps, core_ids=list(range(N_CORES)), trace=trace, tmpdir=tmpdir
    )
    out = np.empty((B, S, DIM), dtype=np.float32)
    for c in range(N_CORES):
        b, g = c // 4, c % 4
        out[b, g * SC:(g + 1) * SC, :] = res.results[c]["out"]
    return out, res


def kernel(**inputs):
    out, _ = run(inputs, trace=False)
    return out
